# revision 7
# baseline (speedup 1.0000x reference)
"""Luong attention decoder RNN (teacher-forced GRU + attention + vocab projection
+ log_softmax) on 8 Trainium2 NeuronCores.

Sharding: data-parallel over batch (B=32 -> 4 per core). Each core runs the
full T=64 recurrence for its 4 batches, then projects its 256 (t,b) rows
against the full vocab with bf16 weights, and normalizes with log-softmax.
No cross-core communication.
"""

import numpy as np

B, S, T, H, V = 32, 64, 64, 1024, 32000
SOS = 0
P = 128
NCORES = 8
BL = B // NCORES          # 4 batches per core
R = T * BL                # 256 rows (t-major: r = t*BL + b)
H3 = 3 * H
KT = H // P               # 8 hidden k-tiles
MT3 = H3 // P             # 24
XKT = (2 * H) // P        # 16 k-tiles of X = [h, ctx]
NCH = 64                  # vocab chunks
CHN = V // NCH            # 500
NG = 8                    # pass-B groups
GW = V // NG              # 4000

_CACHE = {}


def _emit(nc, tc, d):
    import concourse.bass as bass
    import concourse.mybir as mybir

    f32 = mybir.dt.float32
    f32r = mybir.dt.float32r
    bf16 = mybir.dt.bfloat16
    AF = mybir.ActivationFunctionType
    ALU = mybir.AluOpType
    AX = mybir.AxisListType
    PSUM = bass.MemorySpace.PSUM
    dma = nc.sync.dma_start

    import contextlib
    with tc.tile_pool(name="persist", bufs=1) as PP:
        # ---------------- persistent SBUF ----------------
        XT = PP.tile([P, XKT * R], f32r, tag="XT")        # X^T hidden-major
        XT3 = XT[:].rearrange("p (k r) -> p k r", k=XKT)
        XTb = PP.tile([P, XKT * R], bf16, tag="XTb")
        cmax = PP.tile([P, 2 * NCH], f32, tag="cmax")
        csum = PP.tile([P, 2 * NCH], f32, tag="csum")
        negm = PP.tile([P, 2 * NCH], f32, tag="negm")
        off = PP.tile([P, 2], f32, tag="off")

        _ph1_cm = tc.tile_pool(name="ph1", bufs=1)
        PH1 = _ph1_cm.__enter__()
        gi_sb = PH1.tile([P, MT3 * R], f32, tag="gi")
        gi3 = gi_sb[:].rearrange("p (m r) -> p m r", m=MT3)
        keys_sb = PH1.tile([P, KT * 256], f32r, tag="keys")
        enc_sb = PH1.tile([P, 2 * H], f32, tag="enc")
        h0_sb = PH1.tile([P, KT * BL], f32r, tag="h0")
        h03 = h0_sb[:].rearrange("p (k b) -> p k b", k=KT)
        giB = PH1.tile([P, MT3], f32, tag="giB")
        baT = PH1.tile([P, KT], f32, tag="baT")
        id4 = PH1.tile([4, 4], f32, tag="id4")
        mask_sb = PH1.tile([BL, BL * S], f32, tag="mask")
        bhn_sb = PH1.tile([P, KT], f32, tag="bhn")

        dma(h0_sb[:].rearrange("p (k b) -> p k b", k=KT),
            d["h0T"].rearrange("k p b -> p k b"))
        dma(giB[:], d["giBias"])
        dma(baT[:], d["baT"])
        dma(id4[:], d["id4"])
        dma(mask_sb[:], d["mask"])
        dma(bhn_sb[:], d["bhn"])
        dma(enc_sb[:].rearrange("p (k h) -> p k h", k=2),
            d["encBM"].rearrange("k p h -> p k h"))

        # ---------------- precompute: keys ----------------
        with (
            tc.tile_pool(name="wa", bufs=1) as WAP,
            tc.tile_pool(name="preps", bufs=4, space=PSUM) as PRE_PS,
        ):
            encT_sb = WAP.tile([P, KT * 256], f32r, tag="encT")
            dma(encT_sb[:].rearrange("p (k r) -> p k r", k=KT),
                d["encT"].rearrange("k p r -> p k r"))
            wa_chunks = []
            for k in range(KT):
                w = WAP.tile([P, H], f32r, tag=f"wa{k}")
                dma(w[:], d["WaT"][k])
                wa_chunks.append(w)
            # keys^T = Wa @ enc^T + ba   (hidden-major [128, (k_h, b*S)])
            for mt in range(KT):
                ps = PRE_PS.tile([P, 256], f32, tag="pre")
                for k in range(KT):
                    nc.tensor.matmul(
                        ps[:],
                        wa_chunks[k][:, mt * P:(mt + 1) * P],
                        encT_sb[:, k * 256:(k + 1) * 256],
                        start=(k == 0), stop=(k == KT - 1))
                nc.vector.tensor_scalar_add(
                    keys_sb[:, mt * 256:(mt + 1) * 256], ps[:], baT[:, mt:mt + 1])

        # ---------------- precompute: gi ----------------
        with (
            tc.tile_pool(name="wih", bufs=1) as WIP,
            tc.tile_pool(name="preps2", bufs=4, space=PSUM) as PRE_PS2,
        ):
            xts = WIP.tile([P, KT * R], f32r, tag="xts")
            dma(xts[:].rearrange("p (k r) -> p k r", k=KT),
                d["xT"].rearrange("k p r -> p k r"))
            wih_chunks = []
            for k in range(KT):
                w = WIP.tile([P, H3], f32r, tag=f"wih{k}")
                dma(w[:], d["WihT"][k])
                wih_chunks.append(w)
            # gi^T = W_ih @ x^T + (b_ih + b_hh)   (hidden-major [128, (mt, r)])
            for mt in range(MT3):
                ps = PRE_PS2.tile([P, R], f32, tag="pre")
                for k in range(KT):
                    nc.tensor.matmul(
                        ps[:],
                        wih_chunks[k][:, mt * P:(mt + 1) * P],
                        xts[:, k * R:(k + 1) * R],
                        start=(k == 0), stop=(k == KT - 1))
                nc.vector.tensor_scalar_add(
                    gi_sb[:, mt * R:(mt + 1) * R], ps[:], giB[:, mt:mt + 1])

        # ---------------- recurrence ----------------
        with (
            tc.tile_pool(name="whh", bufs=1) as WHP,
            tc.tile_pool(name="ghps", bufs=2, space=PSUM) as GHPS,
            tc.tile_pool(name="tpps", bufs=2, space=PSUM) as TPPS,
            tc.tile_pool(name="atps", bufs=1, space=PSUM) as ATPS,
            tc.tile_pool(name="ghs", bufs=3) as GHS,
            tc.tile_pool(name="gt", bufs=2) as GT,
            tc.tile_pool(name="att", bufs=2) as ATP,
        ):
            whh = WHP.tile([P, KT * H3], f32r, tag="whh")
            dma(whh[:].rearrange("p (k n) -> p k n", k=KT),
                d["WhhT"].rearrange("k p n -> p k n"))

            for t in range(T):
                def h_ap(klo, khi):
                    """h_{t-1}^T tiles [128, khi-klo, 4] (hidden-major)."""
                    if t == 0:
                        return h03[:, klo:khi, :]
                    return XT3[:, klo:khi, BL * (t - 1):BL * (t - 1) + BL]

                def h_ap1(k):
                    if t == 0:
                        return h0_sb[:, k * BL:(k + 1) * BL]
                    return XT3[:, k, BL * (t - 1):BL * (t - 1) + BL]

                # gh = h @ W_hh^T in 6 chunks (gate g, half hh), then PE-transpose
                # each chunk to hidden-major [128, (g, jloc, b)] in PSUM.
                tp_half = []
                for hh in range(2):
                    tp = TPPS.tile([P, 48], f32, tag="tp")
                    for g in range(3):
                        coff = g * H + hh * 512
                        ps = GHPS.tile([BL, 512], f32, tag="gh")
                        for k in range(KT):
                            nc.tensor.matmul(
                                ps[:],
                                h_ap1(k),
                                whh[:, k * H3 + coff:k * H3 + coff + 512],
                                start=(k == 0), stop=(k == KT - 1))
                        ghc = GHS.tile([BL, 512], f32, tag="ghs")
                        nc.vector.tensor_copy(ghc[:], ps[:])
                        for jj in range(4):
                            nc.tensor.transpose(
                                tp[:, g * 16 + jj * 4:g * 16 + jj * 4 + 4],
                                ghc[0:BL, jj * P:(jj + 1) * P],
                                id4[:])
                    tp_half.append(tp)

                # GRU gates per half, hidden-major [128, 4, 4]
                for hh in range(2):
                    tpv = tp_half[hh][:].rearrange("p (g j b) -> p g j b", g=3, j=4)
                    gir = gi3[:, 0 * KT + 4 * hh:0 * KT + 4 * hh + 4, BL * t:BL * t + BL]
                    giz = gi3[:, 1 * KT + 4 * hh:1 * KT + 4 * hh + 4, BL * t:BL * t + BL]
                    gin = gi3[:, 2 * KT + 4 * hh:2 * KT + 4 * hh + 4, BL * t:BL * t + BL]
                    ar = GT.tile([P, 16], f32, tag="ar")
                    az = GT.tile([P, 16], f32, tag="az")
                    rg = GT.tile([P, 16], f32, tag="rg")
                    zg = GT.tile([P, 16], f32, tag="zg")
                    mn = GT.tile([P, 16], f32, tag="mn")
                    mn2 = GT.tile([P, 16], f32, tag="mn2")
                    ng_ = GT.tile([P, 16], f32, tag="ng")
                    dd = GT.tile([P, 16], f32, tag="dd")
                    ee = GT.tile([P, 16], f32, tag="ee")

                    def v3(tile):
                        return tile[:].rearrange("p (j b) -> p j b", j=4)

                    nc.vector.tensor_add(v3(ar), tpv[:, 0], gir)
                    nc.vector.tensor_add(v3(az), tpv[:, 1], giz)
                    nc.scalar.activation(rg[:], ar[:], AF.Sigmoid)
                    nc.scalar.activation(zg[:], az[:], AF.Sigmoid)
                    bhn_b = bhn_sb[:, 4 * hh:4 * hh + 4].unsqueeze(2)                         .broadcast_to([P, 4, BL])
                    ghn = GT.tile([P, 16], f32, tag="ghn")
                    nc.vector.tensor_add(v3(ghn), tpv[:, 2], bhn_b)
                    nc.vector.tensor_mul(v3(mn), v3(ghn), v3(rg))
                    nc.vector.tensor_add(v3(mn2), v3(mn), gin)
                    nc.scalar.activation(ng_[:], mn2[:], AF.Tanh)
                    nc.vector.tensor_sub(v3(dd), h_ap(4 * hh, 4 * hh + 4), v3(ng_))
                    nc.vector.tensor_mul(ee[:], zg[:], dd[:])
                    nc.vector.tensor_add(
                        XT3[:, 4 * hh:4 * hh + 4, BL * t:BL * t + BL],
                        v3(ng_), v3(ee))

                # ---- attention ----
                # scores for all (b', (b,s)) pairs; mask off-diagonal b' != b
                # with -1e9 so the row softmax directly yields the
                # block-diagonal attention matrix A [4, 256].
                sc_ps = ATPS.tile([BL, BL * S], f32, tag="sc")
                for k in range(KT):
                    nc.tensor.matmul(
                        sc_ps[:],
                        XT3[:, k, BL * t:BL * t + BL],
                        keys_sb[:, k * 256:(k + 1) * 256],
                        start=(k == 0), stop=(k == KT - 1))
                scm = ATP.tile([BL, BL * S], f32, tag="scm")
                nc.vector.tensor_add(scm[:], sc_ps[:], mask_sb[:])
                nmx = ATP.tile([BL, 1], f32, tag="nmx")
                nc.vector.tensor_reduce(nmx[:], scm[:], axis=AX.X, op=ALU.max,
                                        negate=True)
                ssum = ATP.tile([BL, 1], f32, tag="ssum")
                ae = ATP.tile([BL, BL * S], f32, tag="ae")
                nc.scalar.activation(ae[:], scm[:], AF.Exp, bias=nmx[:],
                                     accum_out=ssum[:])
                rin = ATP.tile([BL, 1], f32, tag="rin")
                nc.vector.reciprocal(rin[:], ssum[:])
                attn_t = ATP.tile([BL, BL * S], f32, tag="attn_t")
                nc.vector.tensor_scalar_mul(attn_t[:], ae[:], rin[:])
                for b in range(BL):
                    nc.sync.dma_start(d["attn_o"][b, t, :],
                                      attn_t[b:b + 1, b * S:(b + 1) * S])

                # A^T [256, 4] as [128, (kk, b')] via two PE transposes
                at_ps = ATPS.tile([P, 2 * BL], f32, tag="at")
                for kk in range(2):
                    nc.tensor.transpose(
                        at_ps[:, kk * BL:(kk + 1) * BL],
                        attn_t[0:BL, kk * P:(kk + 1) * P], id4[:])
                AT = ATP.tile([P, 2 * BL], f32, tag="AT")
                nc.vector.tensor_copy(AT[:], at_ps[:])
                # ctx^T = enc_flat^T @ A^T  (hidden-major [128, (mj, b)])
                ctx_ps = ATPS.tile([P, KT * BL], f32, tag="ctx")
                for mj in range(KT):
                    for kk in range(2):
                        nc.tensor.matmul(
                            ctx_ps[:, mj * BL:(mj + 1) * BL],
                            enc_sb[:, kk * H + mj * P:kk * H + (mj + 1) * P],
                            AT[:, kk * BL:kk * BL + BL],
                            start=(kk == 0), stop=(kk == 1))
                nc.vector.tensor_copy(
                    XT3[:, KT:2 * KT, BL * t:BL * t + BL],
                    ctx_ps[:].rearrange("p (m b) -> p m b", m=KT))

            # h_final out
            dma(d["hfin"].rearrange("k p b -> p k b"),
                XT3[:, 0:KT, BL * (T - 1):BL * (T - 1) + BL].bitcast(f32))

        _ph1_cm.__exit__(None, None, None)

        # ---------------- phase 2: logits + stats (raw logits -> out dram) ------
        nc.vector.tensor_copy(XTb[:], XT[:].bitcast(f32))
        with (
            tc.tile_pool(name="wo", bufs=3) as WOP,
            tc.tile_pool(name="boP", bufs=1) as BOP,
            tc.tile_pool(name="stg", bufs=4) as STG,
            tc.tile_pool(name="esc", bufs=2) as ESC,
            tc.tile_pool(name="p2ps", bufs=4, space=PSUM) as P2PS,
        ):
            ones1 = BOP.tile([1, P], bf16, tag="ones1")
            nc.vector.memset(ones1[:], 1.0)
            bo_sb = BOP.tile([1, V], bf16, tag="bo")
            dma(bo_sb[:], d["bo"])
            for c in range(NCH):
                wt = WOP.tile([P, XKT, CHN], bf16, tag="wo")
                dma(wt[:], d["Wo"][c])
                for m in range(2):
                    ps = P2PS.tile([P, CHN], f32, tag="p2")
                    for k in range(XKT):
                        nc.tensor.matmul(
                            ps[:],
                            XTb[:, k * R + m * P:k * R + (m + 1) * P],
                            wt[:, k, :],
                            start=(k == 0), stop=False)
                    nc.tensor.matmul(
                        ps[:], ones1[:], bo_sb[0:1, c * CHN:(c + 1) * CHN],
                        start=False, stop=True)
                    stg = STG.tile([P, CHN], f32, tag="stg")
                    nc.vector.tensor_copy(stg[:], ps[:])
                    i = m * NCH + c
                    nc.vector.tensor_reduce(
                        cmax[:, i:i + 1], stg[:], axis=AX.X, op=ALU.max)
                    nc.vector.tensor_scalar_mul(
                        negm[:, i:i + 1], cmax[:, i:i + 1], -1.0)
                    esc = ESC.tile([P, CHN], f32, tag="esc")
                    nc.scalar.activation(
                        esc[:], stg[:], AF.Exp, bias=negm[:, i:i + 1],
                        accum_out=csum[:, i:i + 1])
                    dma(d["outP"][m * P:(m + 1) * P, c * CHN:(c + 1) * CHN], stg[:])

            # final log-sum-exp stats per m-tile
            for m in range(2):
                mM = STG.tile([P, 1], f32, tag="mM")
                nc.vector.tensor_reduce(
                    mM[:], cmax[:, m * NCH:(m + 1) * NCH], axis=AX.X, op=ALU.max)
                nmM = STG.tile([P, 1], f32, tag="nmM")
                nc.vector.tensor_scalar_mul(nmM[:], mM[:], -1.0)
                t1 = STG.tile([P, NCH], f32, tag="t1")
                nc.scalar.activation(
                    t1[:], cmax[:, m * NCH:(m + 1) * NCH], AF.Exp, bias=nmM[:])
                t2 = STG.tile([P, NCH], f32, tag="t2")
                nc.vector.tensor_mul(t2[:], t1[:], csum[:, m * NCH:(m + 1) * NCH])
                sS = STG.tile([P, 1], f32, tag="sS")
                nc.vector.tensor_reduce(sS[:], t2[:], axis=AX.X, op=ALU.add)
                lnS = STG.tile([P, 1], f32, tag="lnS")
                nc.scalar.activation(lnS[:], sS[:], AF.Ln)
                nc.vector.tensor_add(off[:, m:m + 1], mM[:], lnS[:])

        # ---------------- pass B: log_probs = logits - off, in place ----------
        with tc.tile_pool(name="pb", bufs=2) as PB:
            for m in range(2):
                for g in range(NG):
                    bi = PB.tile([P, GW], f32, tag="bi")
                    dma(bi[:], d["outP"][m * P:(m + 1) * P, g * GW:(g + 1) * GW])
                    bo_ = PB.tile([P, GW], f32, tag="bo_")
                    nc.vector.tensor_scalar_sub(bo_[:], bi[:], off[:, m:m + 1])
                    dma(d["outP"][m * P:(m + 1) * P, g * GW:(g + 1) * GW], bo_[:])


def _build():
    if "nc" in _CACHE:
        return _CACHE["nc"]
    import concourse.bacc as bacc
    import concourse.tile as tile
    import concourse.mybir as mybir

    f32 = mybir.dt.float32
    f32r = mybir.dt.float32r
    bf16 = mybir.dt.bfloat16
    nc = bacc.Bacc("TRN2", target_bir_lowering=False, debug=False,
                   num_devices=NCORES)

    def din(name, shape, dt=f32):
        return nc.dram_tensor(name, shape, dt, kind="ExternalInput").ap()

    def dout(name, shape, dt=f32):
        return nc.dram_tensor(name, shape, dt, kind="ExternalOutput").ap()

    d = {
        "xT": din("xT", [KT, P, R], f32r),
        "encT": din("encT", [KT, P, 256], f32r),
        "encBM": din("encBM", [2, P, H]),
        "h0T": din("h0T", [KT, P, BL], f32r),
        "WihT": din("WihT", [KT, P, H3], f32r),
        "WhhT": din("WhhT", [KT, P, H3], f32r),
        "WaT": din("WaT", [KT, P, H], f32r),
        "giBias": din("giBias", [P, MT3]),
        "baT": din("baT", [P, KT]),
        "Wo": din("Wo", [NCH, P, XKT, CHN], bf16),
        "bo": din("bo", [1, V], bf16),
        "id4": din("id4", [4, 4]),
        "mask": din("mask", [BL, BL * S]),
        "bhn": din("bhn", [P, KT]),
        "outP": dout("outP", [R, V]),
        "hfin": dout("hfin", [KT, P, BL]),
        "attn_o": dout("attn_o", [BL, T, S]),
    }
    with tile.TileContext(nc) as tc:
        _emit(nc, tc, d)
    nc.compile()
    _CACHE["nc"] = nc
    return nc


def _host_prep(inputs):
    import ml_dtypes
    bf16 = ml_dtypes.bfloat16
    f32 = np.float32

    enc = np.asarray(inputs["encoder_outputs"], f32)
    h0 = np.asarray(inputs["encoder_hidden"], f32)
    tgt = np.asarray(inputs["target_tensor"])
    emb = np.asarray(inputs["embedding"], f32)
    W_ih = np.asarray(inputs["W_ih"], f32)
    W_hh = np.asarray(inputs["W_hh"], f32)
    b_ih = np.asarray(inputs["b_ih"], f32)
    b_hh = np.asarray(inputs["b_hh"], f32)
    Wa = np.asarray(inputs["Wa"], f32)
    ba = np.asarray(inputs["ba"], f32)
    W_out = np.asarray(inputs["W_out"], f32)
    b_out = np.asarray(inputs["b_out"], f32)

    idx = np.concatenate([np.full((B, 1), SOS, tgt.dtype), tgt[:, :-1]], axis=1)
    x = emb[idx]  # [B, T, H]

    WihT = np.ascontiguousarray(W_ih.T).reshape(KT, P, H3)
    WhhT = np.ascontiguousarray(W_hh.T).reshape(KT, P, H3)
    WaT = np.ascontiguousarray(Wa.T).reshape(KT, P, H)
    bias_sum = b_ih + b_hh
    bias_sum[2 * H:] = b_ih[2 * H:]          # b_hn applies inside the r gate
    giBias = np.ascontiguousarray(bias_sum.reshape(MT3, P).T)
    bhn = np.ascontiguousarray(b_hh[2 * H:].reshape(KT, P).T)
    baT = np.ascontiguousarray(ba.reshape(KT, P).T)
    Wo = np.ascontiguousarray(
        np.ascontiguousarray(W_out.T).reshape(XKT, P, NCH, CHN)
        .transpose(2, 1, 0, 3)).astype(bf16)
    bo = b_out.astype(bf16)[None, :]
    id4 = np.eye(4, dtype=f32)
    mask = np.full((BL, BL * S), -1e9, f32)
    for b in range(BL):
        mask[b, b * S:(b + 1) * S] = 0.0

    shared = dict(WihT=WihT, WhhT=WhhT, WaT=WaT, giBias=giBias, baT=baT,
                  Wo=Wo, bo=bo, id4=id4, mask=mask, bhn=bhn)

    in_maps = []
    for c in range(NCORES):
        bsl = slice(BL * c, BL * (c + 1))
        xc = x[bsl]                      # [4, T, H]
        xT = np.ascontiguousarray(
            xc.transpose(1, 0, 2).reshape(R, H).T).reshape(KT, P, R)
        ec = np.ascontiguousarray(enc[bsl])  # [4, S, H]
        encT = np.ascontiguousarray(
            ec.reshape(BL * S, H).T).reshape(KT, P, 256)
        encBM = ec.reshape(2, P, H)
        h0T = np.ascontiguousarray(h0[bsl].T).reshape(KT, P, BL)
        m = dict(shared)
        m.update(xT=xT, encT=encT, encBM=encBM, h0T=h0T)
        in_maps.append(m)
    return in_maps


def run_cores(inputs, trace=False):
    """Returns BassKernelResults (results = per-core dicts of outputs)."""
    from concourse import bass_utils
    nc = _build()
    in_maps = _host_prep(inputs)
    res = bass_utils.run_bass_kernel_spmd(
        nc, in_maps, core_ids=list(range(NCORES)), trace=trace)
    return res


def kernel(**inputs):
    res = run_cores(inputs, trace=False)
    outs = res.results
    lp = np.stack([np.asarray(o["outP"], np.float32).reshape(T, BL, V)
                   .transpose(1, 0, 2) for o in outs])
    log_probs = lp.reshape(B, T, V)
    h_final = np.concatenate(
        [np.asarray(o["hfin"], np.float32).reshape(H, BL).T for o in outs], axis=0)
    attns = np.concatenate(
        [np.asarray(o["attn_o"], np.float32).reshape(BL, T, S) for o in outs],
        axis=0)
    return log_probs, h_final, attns
